# revision 21
# baseline (speedup 1.0000x reference)
"""MultiHeadAttention Trainium2 kernel (8 NeuronCores).

Sharding: data-parallel over batch (2) x tensor-parallel over heads (16/4=4
head groups). Core c handles batch b = c//4 and heads 4g..4g+4 (g = c%4),
i.e. a 256-wide column slice of Wq/Wk/Wv and the matching row slice of Wo.
Each core computes a full [2048, 1024] partial output (its heads' ctx @ Wo
row-slice); the host sums the 4 partials per batch and adds the bias terms.

v3: fine-grained software pipeline. The kernel interleaves scores+exp units
(2 matmuls + 1 ACT) with projection / ctx / out-proj chain fillers at ~1 us
granularity so the tensor engine never idles (PE p-state ramp makes every
idle gap cost double) and the scalar engine (exp, ~143 us total) always has
backlog. Host pre-arranges x and weights into the exact SBUF tile layouts so
every DMA is large contiguous descriptors, ordered by first use. Output is
written fp16 (host accumulates partials in f32). 1/sqrt(dk) folded into
Wq/bq.

Per-core dataflow (all matmul operands fp16, fp32 PSUM):
  Q.T, K.T = W.T @ xT + b (per-partition bias)     [d'=256, s] pair-packed
  V        = xT.T @ WvT (no bias; folded on host)  [s, c] + ones col/head
  scores.T = K.T_h.T @ Q.T_h (per 256-wide k slab) [k, q] in PSUM
  P.T      = exp(scores.T) on ACT, fp16            [k, q] SBUF
  ctx.T|r  = [V_h | 1].T @ P.T (M=65, fused rowsum), 16-kt chain in PSUM
  ctx_n    = ctx.T * broadcast(1/r)                [c, q] pair-packed fp16
  out_u    = ctx_n.T @ WoT                         [s, 1024] -> DRAM fp16
"""

import numpy as np

import concourse.bass as bass
import concourse.mybir as mybir
import concourse.tile as tile
from concourse import bacc
from concourse.bass_utils import run_bass_kernel_spmd

S = 2048          # sequence length
D = 1024          # model dim
DC = 256          # d' columns per core (4 heads x 64)
H = 4             # heads per core
DK = 64           # head dim
P = 128
F32 = mybir.dt.float32
FP16 = mybir.dt.float16
NCORES = 8

_cached = {}


def build_program():
    nc = bacc.Bacc("TRN2", target_bir_lowering=False, debug=False,
                   num_devices=NCORES)

    # x pre-arranged on host: xp[p, 8*sc + t, s] = x.T[128*t + p, 512*sc + s]
    xq = nc.dram_tensor("xq", [P, 32, 512], FP16, kind="ExternalInput").ap()
    xk = nc.dram_tensor("xk", [P, 32, 512], FP16, kind="ExternalInput").ap()
    xv = nc.dram_tensor("xv", [P, 32, 512], FP16, kind="ExternalInput").ap()
    # weights pre-arranged: w[p, t, c] = W.T[128*t + p, c]
    wq = nc.dram_tensor("wq", [P, 8, DC], FP16, kind="ExternalInput").ap()
    wk = nc.dram_tensor("wk", [P, 8, DC], FP16, kind="ExternalInput").ap()
    wv = nc.dram_tensor("wv", [P, 8, DC], FP16, kind="ExternalInput").ap()
    wo = nc.dram_tensor("wo", [P, 2, D], FP16, kind="ExternalInput").ap()
    bq = nc.dram_tensor("bq", [P, 2], F32, kind="ExternalInput").ap()
    bk = nc.dram_tensor("bk", [P, 2], F32, kind="ExternalInput").ap()
    out = nc.dram_tensor("out", [S, D], FP16, kind="ExternalOutput").ap()

    with tile.TileContext(nc) as tc:
        build_tile_kernel(nc, tc, xq, xk, xv, wq, wk, wv, wo, bq, bk, out)

    nc.compile()
    return nc


def build_tile_kernel(nc, tc, xq, xk, xv, wq, wk, wv, wo, bq, bk, out):
    from contextlib import ExitStack

    with ExitStack() as ctx:
        singles = ctx.enter_context(tc.tile_pool(name="singles", bufs=1))
        persist = ctx.enter_context(tc.tile_pool(name="persist", bufs=1))
        # PSUM: psA = scores slots (2 banks each), psB = chain slots (1 bank)
        psA = ctx.enter_context(tc.tile_pool(name="psA", bufs=3, space="PSUM"))
        psB = ctx.enter_context(tc.tile_pool(name="psB", bufs=2, space="PSUM"))
        xT_pool = ctx.enter_context(tc.tile_pool(name="xT", bufs=8))
        pT_pool = ctx.enter_context(tc.tile_pool(name="pT", bufs=6))
        norm_pool = ctx.enter_context(tc.tile_pool(name="norm", bufs=2))
        out_sb_pool = ctx.enter_context(tc.tile_pool(name="osb", bufs=2))

        # --- weight / bias tiles -------------------------------------------------
        w_q = singles.tile([P, 8, DC], FP16, tag="w_q")
        w_k = singles.tile([P, 8, DC], FP16, tag="w_k")
        w_v = singles.tile([P, 8, DC], FP16, tag="w_v")
        w_o = singles.tile([P, 2, D], FP16, tag="w_o")
        bq_t = singles.tile([P, 2], F32, tag="bq")
        bk_t = singles.tile([P, 2], F32, tag="bk")

        # --- persistent activations ---------------------------------------------
        qT = persist.tile([P, 2, S], FP16, tag="qT")    # [d'%128, pair, s]
        kT = persist.tile([P, 2, S], FP16, tag="kT")
        v_sb = persist.tile([P, 16, H * (DK + 1)], FP16, tag="v_sb")
        ctxn = persist.tile([P, 2, S], FP16, tag="ctxn")  # [c%128, pair, q]

        # --- DMA emission, ordered by first use ----------------------------------
        x_tiles = {}

        def dma_x(name, dram, sc, half):
            t = xT_pool.tile([P, 4, 512], FP16, tag="xc",
                             name=f"x{name}{sc}{half}")
            nc.sync.dma_start(
                out=t, in_=dram[:, 8 * sc + 4 * half:8 * sc + 4 * half + 4, :])
            x_tiles[(name, sc, half)] = t

        nc.sync.dma_start(out=bq_t, in_=bq)
        nc.sync.dma_start(out=bk_t, in_=bk)
        nc.sync.dma_start(out=w_k, in_=wk)
        dma_x("k", xk, 0, 0)
        dma_x("k", xk, 0, 1)
        nc.sync.dma_start(out=w_q, in_=wq)
        dma_x("q", xq, 0, 0)
        dma_x("q", xq, 0, 1)
        dma_x("k", xk, 1, 0)
        dma_x("k", xk, 1, 1)
        dma_x("k", xk, 2, 0)
        dma_x("k", xk, 2, 1)
        dma_x("k", xk, 3, 0)
        dma_x("k", xk, 3, 1)
        nc.sync.dma_start(out=w_v, in_=wv)
        dma_x("v", xv, 0, 0)
        dma_x("v", xv, 0, 1)
        dma_x("v", xv, 1, 0)
        dma_x("v", xv, 1, 1)
        dma_x("q", xq, 1, 0)
        dma_x("q", xq, 1, 1)
        dma_x("v", xv, 2, 0)
        dma_x("v", xv, 2, 1)
        dma_x("v", xv, 3, 0)
        dma_x("v", xv, 3, 1)
        nc.sync.dma_start(out=w_o, in_=wo)
        dma_x("q", xq, 2, 0)
        dma_x("q", xq, 2, 1)
        dma_x("q", xq, 3, 0)
        dma_x("q", xq, 3, 1)

        for h in range(H):  # ones column per head for rowsum-in-matmul
            nc.vector.memset(v_sb[:, :, h * 65 + 64:h * 65 + 65], 1.0)

        # --- unit emitters -------------------------------------------------------
        proj_accs = {}

        def qk_half(name, w_t, b_t, dest, sc, m, lo):
            # half of a Q/K projection m-chain: 4 matmuls N=512
            if lo == 0:
                pr = psB.tile([P, 512], F32, tag="ps1",
                              name=f"pr_{name}_{sc}_{m}")
                proj_accs[(name, sc, m)] = pr
            else:
                pr = proj_accs.pop((name, sc, m))
            for dt in range(4 * lo, 4 * lo + 4):
                xc = x_tiles[(name, sc, dt // 4)]
                nc.tensor.matmul(
                    pr,
                    lhsT=w_t[:, dt, 128 * m:128 * (m + 1)],
                    rhs=xc[:, dt % 4, :],
                    start=(dt == 0), stop=(dt == 7))
            if lo:
                nc.vector.tensor_scalar_add(
                    dest[:, m, 512 * sc:512 * (sc + 1)], pr, b_t[:, m:m + 1])

        def qk_proj(name, w_t, b_t, dest, sc, m):
            qk_half(name, w_t, b_t, dest, sc, m, 0)
            qk_half(name, w_t, b_t, dest, sc, m, 1)

        def v_proj(sc, st):
            # one st-chain of the V projection: 8 matmuls N=256
            pv = psB.tile([P, DC], F32, tag="ps1", name=f"pv_{sc}_{st}")
            for dt in range(8):
                xc = x_tiles[("v", sc, dt // 4)]
                nc.tensor.matmul(
                    pv,
                    lhsT=xc[:, dt % 4, 128 * st:128 * (st + 1)],
                    rhs=w_v[:, dt, :],
                    start=(dt == 0), stop=(dt == 7))
            kt = 4 * sc + st
            nc.vector.tensor_copy(
                v_sb[:, kt, :].rearrange("p (h c) -> p h c", h=H)[:, :, 0:DK],
                pv.rearrange("p (h c) -> p h c", c=DK))

        pT_tiles = {}

        def s_unit(qc, pr_i, half, kg):
            # scores for one head over a 256-wide k slab + exp: 2 mm + 1 ACT
            h = 2 * pr_i + half
            if (qc, h) not in pT_tiles:
                pT_tiles[(qc, h)] = pT_pool.tile(
                    [P, 16, 512], FP16, tag="pT", name=f"pT_{qc}_{h}")
            pT_h = pT_tiles[(qc, h)]
            qsl = slice(512 * qc, 512 * (qc + 1))
            rows = slice(64 * half, 64 * half + 64)
            sc_ = psA.tile([P, 2, 512], F32, tag="sc",
                           name=f"sc_{qc}_{pr_i}_{half}_{kg}")
            for khi in range(2):
                kt = 2 * kg + khi
                nc.tensor.matmul(sc_[:, khi, :],
                                 lhsT=kT[rows, pr_i, 128 * kt:128 * (kt + 1)],
                                 rhs=qT[rows, pr_i, qsl])
            nc.scalar.activation(
                pT_h[:, 2 * kg:2 * kg + 2, :].rearrange("p a b -> p (a b)"),
                sc_.rearrange("p a b -> p (a b)"),
                mybir.ActivationFunctionType.Exp)

        ctx_accs = {}

        def ctx_half(qc, h, lo):
            # half of a ctx chain: 8 matmuls; second half finishes + normalizes
            qsl = slice(512 * qc, 512 * (qc + 1))
            pr_i, hp = divmod(h, 2)
            if lo == 0:
                acc = psB.tile([P, 512], F32, tag="ps1", name=f"cp_{qc}_{h}")
                ctx_accs[(qc, h)] = acc
            else:
                acc = ctx_accs.pop((qc, h))
            pT_h = pT_tiles[(qc, h)]
            for kt in range(8 * lo, 8 * lo + 8):
                nc.tensor.matmul(
                    acc[0:65, :],
                    lhsT=v_sb[:, kt, 65 * h:65 * h + 65],
                    rhs=pT_h[:, kt, :],
                    start=(kt == 0), stop=(kt == 15))
            if lo == 0:
                return
            del pT_tiles[(qc, h)]
            # normalize: ctx_n = ctx * bcast(1/rowsum)
            rs = norm_pool.tile([1, 512], F32, tag="rs", name=f"rs_{qc}_{h}")
            nc.vector.tensor_copy(rs, acc[64:65, :])
            rc = norm_pool.tile([1, 512], F32, tag="rc", name=f"rc_{qc}_{h}")
            nc.vector.reciprocal_approx_fast(rc, rs)
            bc = norm_pool.tile([64, 512], F32, tag="bc", name=f"bc_{qc}_{h}")
            nc.gpsimd.partition_broadcast(bc, rc[0:1, :], channels=64)
            nc.vector.tensor_mul(
                ctxn[64 * hp:64 * hp + 64, pr_i, qsl], acc[0:64, :], bc)

        def outproj(qc, stl):
            st = 4 * qc + stl
            ob = out_sb_pool.tile([P, D], FP16, tag="ob", name=f"ob_{st}")
            for jc in range(2):
                op = psB.tile([P, 512], F32, tag="ps1", name=f"op_{st}_{jc}")
                for ct in range(2):
                    nc.tensor.matmul(
                        op,
                        lhsT=ctxn[:, ct, 128 * st:128 * (st + 1)],
                        rhs=w_o[:, ct, 512 * jc:512 * (jc + 1)],
                        start=(ct == 0), stop=(ct == 1))
                nc.vector.tensor_copy(ob[:, 512 * jc:512 * (jc + 1)], op)
            nc.sync.dma_start(out=out[128 * st:128 * (st + 1), :], in_=ob)

        # --- schedule ------------------------------------------------------------
        # prologue: K chunk 0 + Q chunk 0 pair 0 (minimum for first S unit)
        qk_proj("k", w_k, bk_t, kT, 0, 0)
        qk_proj("k", w_k, bk_t, kT, 0, 1)
        qk_proj("q", w_q, bq_t, qT, 0, 0)

        # filler inventory per 16-tick block, (cycles, emit) pairs.
        # CT halves must stay adjacent (open PSUM chain; only S units between).
        def F(cyc, fn, *a):
            return (cyc, lambda: fn(*a))

        def KPu(sc, m):
            return [F(2048, qk_half, "k", w_k, bk_t, kT, sc, m, 0),
                    F(2048, qk_half, "k", w_k, bk_t, kT, sc, m, 1)]

        def QPu(sc, m):
            return [F(2048, qk_half, "q", w_q, bq_t, qT, sc, m, 0),
                    F(2048, qk_half, "q", w_q, bq_t, qT, sc, m, 1)]

        def VPu(sc, st):
            return F(2048, v_proj, sc, st)

        def CTu(qc, h):
            return [F(4096, ctx_half, qc, h, 0), F(4096, ctx_half, qc, h, 1)]

        def OPu(qc, stl):
            return F(2048, outproj, qc, stl)

        blocks = [
            # b0 (qc0,pr0)
            KPu(1, 0) + KPu(1, 1) + KPu(2, 0) + KPu(2, 1) + KPu(3, 0)
            + KPu(3, 1) + QPu(0, 1),
            # b1 (qc0,pr1)
            [VPu(0, st) for st in range(4)] + QPu(1, 0) + QPu(1, 1)
            + [VPu(1, st) for st in range(4)],
            # b2 (qc1,pr0)
            [VPu(2, st) for st in range(4)] + [VPu(3, st) for st in range(4)]
            + CTu(0, 0),
            # b3 (qc1,pr1)
            CTu(0, 1) + CTu(0, 2) + CTu(0, 3) + QPu(2, 0) + QPu(2, 1),
            # b4 (qc2,pr0)
            [OPu(0, stl) for stl in range(4)] + CTu(1, 0) + CTu(1, 1)
            + CTu(1, 2),
            # b5 (qc2,pr1)
            CTu(1, 3) + CTu(2, 0) + [OPu(1, stl) for stl in range(4)]
            + QPu(3, 0),
            # b6 (qc3,pr0)
            QPu(3, 1) + CTu(2, 1) + CTu(2, 2) + CTu(2, 3),
            # b7 (qc3,pr1): S units run all a-halves then all b-halves
            [OPu(2, stl) for stl in range(4)] + CTu(3, 0) + CTu(3, 1)
            + CTu(3, 2),
        ]

        # global pacer: one continuous S stream; leftover fillers flow across
        # block boundaries so no burst ever starves the scalar engine of exp.
        queue = []
        done = 0
        cum = 0
        for bi in range(8):
            qc, pr_i = divmod(bi, 2)
            queue.extend(blocks[bi])
            btotal = sum(c for c, _ in blocks[bi])
            for lt in range(16):
                if bi == 7:
                    half, kg = divmod(lt, 8)
                else:
                    kg, half = divmod(lt, 2)
                s_unit(qc, pr_i, half, kg)
                target = cum + btotal * min(lt + 2, 16) // 16
                while queue and done < target:
                    cyc, fn = queue.pop(0)
                    fn()
                    done += cyc
            cum += btotal
        for cyc, fn in queue:
            fn()

        # epilogue: last head, with norm + outproj pipelined by q-halves
        ctx_half(3, 3, 0)
        acc = ctx_accs.pop((3, 3))
        pT_h = pT_tiles.pop((3, 3))
        for kt in range(8, 16):
            nc.tensor.matmul(
                acc[0:65, :],
                lhsT=v_sb[:, kt, 65 * 3:65 * 3 + 65],
                rhs=pT_h[:, kt, :],
                start=False, stop=(kt == 15))
        rs = norm_pool.tile([1, 512], F32, tag="rs", name="rs_3_3")
        nc.vector.tensor_copy(rs, acc[64:65, :])
        rc = norm_pool.tile([1, 512], F32, tag="rc", name="rc_3_3")
        nc.vector.reciprocal_approx_fast(rc, rs)
        bc = norm_pool.tile([64, 512], F32, tag="bc", name="bc_3_3")
        nc.gpsimd.partition_broadcast(bc, rc[0:1, :], channels=64)
        qb = 512 * 3
        nc.vector.tensor_mul(
            ctxn[64:128, 1, qb:qb + 256], acc[0:64, 0:256], bc[:, 0:256])
        outproj(3, 0)
        outproj(3, 1)
        nc.vector.tensor_mul(
            ctxn[64:128, 1, qb + 256:qb + 512], acc[0:64, 256:512],
            bc[:, 256:512])
        outproj(3, 2)
        outproj(3, 3)


def make_in_maps(Q_input, K_input, V_input, Wq, bq, Wk, bk, Wv, Wo):
    scale = 0.125  # 1/sqrt(64), exact power of two

    def prep_x(x):
        # [S, D] -> xp[p, 8*sc + t, s] = x.T[128*t + p, 512*sc + s]
        xt = x.T.astype(np.float16)                    # [1024, 2048]
        a = xt.reshape(8, P, 4, 512)                   # [t, p, sc, s]
        return np.ascontiguousarray(a.transpose(1, 2, 0, 3)).reshape(P, 32, 512)

    def prep_w(wt):
        # [1024, DC] -> [p, t, c]
        return np.ascontiguousarray(
            wt.reshape(8, P, -1).transpose(1, 0, 2)).astype(np.float16)

    xp = {}
    for b in range(2):
        xp[("q", b)] = prep_x(Q_input[b])
        xp[("k", b)] = prep_x(K_input[b])
        xp[("v", b)] = prep_x(V_input[b])
    in_maps = []
    for c in range(NCORES):
        b, g = divmod(c, 4)
        sl = slice(DC * g, DC * (g + 1))
        in_maps.append({
            "xq": xp[("q", b)],
            "xk": xp[("k", b)],
            "xv": xp[("v", b)],
            "wq": prep_w(np.ascontiguousarray(Wq[sl, :].T) * scale),
            "wk": prep_w(np.ascontiguousarray(Wk[sl, :].T)),
            "wv": prep_w(np.ascontiguousarray(Wv[sl, :].T)),
            "wo": np.ascontiguousarray(
                Wo[:, sl].T.reshape(2, P, D).transpose(1, 0, 2)
            ).astype(np.float16),
            "bq": (bq[sl] * scale).reshape(2, P).T.astype(np.float32).copy(),
            "bk": bk[sl].reshape(2, P).T.astype(np.float32).copy(),
        })
    return in_maps


def kernel(Q_input, K_input, V_input, Wq, bq, Wk, bk, Wv, bv, Wo, bo):
    if "nc" not in _cached:
        _cached["nc"] = build_program()
    nc = _cached["nc"]

    in_maps = make_in_maps(Q_input, K_input, V_input, Wq, bq, Wk, bk, Wv, Wo)
    res = run_bass_kernel_spmd(nc, in_maps, list(range(NCORES))).results
    outs = [res[c]["out"] for c in range(NCORES)]

    const = (bv.astype(np.float32) @ Wo.T.astype(np.float32)) + bo
    full = np.empty((2, S, D), np.float32)
    for b in range(2):
        acc = outs[4 * b].astype(np.float32)
        for g in range(1, 4):
            acc += outs[4 * b + g].astype(np.float32)
        full[b] = acc + const
    return full


# revision 27
# speedup vs baseline: 1.0065x; 1.0065x over previous
"""MultiHeadAttention Trainium2 kernel (8 NeuronCores).

Sharding: data-parallel over batch (2) x tensor-parallel over heads (16/4=4
head groups). Core c handles batch b = c//4 and heads 4g..4g+4 (g = c%4),
i.e. a 256-wide column slice of Wq/Wk/Wv and the matching row slice of Wo.
Each core computes a full [2048, 1024] partial output (its heads' ctx @ Wo
row-slice); the host sums the 4 partials per batch and adds the bias terms.

v3: fine-grained software pipeline. The kernel interleaves scores+exp units
(2 matmuls + 1 ACT) with projection / ctx / out-proj chain fillers at ~1 us
granularity so the tensor engine never idles (PE p-state ramp makes every
idle gap cost double) and the scalar engine (exp, ~143 us total) always has
backlog. Host pre-arranges x and weights into the exact SBUF tile layouts so
every DMA is large contiguous descriptors, ordered by first use. Output is
written fp16 (host accumulates partials in f32). 1/sqrt(dk) folded into
Wq/bq.

Per-core dataflow (all matmul operands fp16, fp32 PSUM):
  Q.T, K.T = W.T @ xT + b (per-partition bias)     [d'=256, s] pair-packed
  V        = xT.T @ WvT (no bias; folded on host)  [s, c] + ones col/head
  scores.T = K.T_h.T @ Q.T_h (per 256-wide k slab) [k, q] in PSUM
  P.T      = exp(scores.T) on ACT, fp16            [k, q] SBUF
  ctx.T|r  = [V_h | 1].T @ P.T (M=65, fused rowsum), 16-kt chain in PSUM
  ctx_n    = ctx.T * broadcast(1/r)                [c, q] pair-packed fp16
  out_u    = ctx_n.T @ WoT                         [s, 1024] -> DRAM fp16
"""

import numpy as np

import concourse.bass as bass
import concourse.mybir as mybir
import concourse.tile as tile
from concourse import bacc
from concourse.bass_utils import run_bass_kernel_spmd

S = 2048          # sequence length
D = 1024          # model dim
DC = 256          # d' columns per core (4 heads x 64)
H = 4             # heads per core
DK = 64           # head dim
P = 128
F32 = mybir.dt.float32
FP16 = mybir.dt.float16
NCORES = 8

_cached = {}


def build_program():
    nc = bacc.Bacc("TRN2", target_bir_lowering=False, debug=False,
                   num_devices=NCORES)

    # x pre-arranged on host: xp[p, 8*sc + t, s] = x.T[128*t + p, 512*sc + s]
    xq = nc.dram_tensor("xq", [P, 32, 512], FP16, kind="ExternalInput").ap()
    xk = nc.dram_tensor("xk", [P, 32, 512], FP16, kind="ExternalInput").ap()
    xv = nc.dram_tensor("xv", [P, 32, 512], FP16, kind="ExternalInput").ap()
    # weights pre-arranged: w[p, t, c] = W.T[128*t + p, c]
    wq = nc.dram_tensor("wq", [P, 8, DC], FP16, kind="ExternalInput").ap()
    wk = nc.dram_tensor("wk", [P, 8, DC], FP16, kind="ExternalInput").ap()
    wv = nc.dram_tensor("wv", [P, 8, DC], FP16, kind="ExternalInput").ap()
    wo = nc.dram_tensor("wo", [P, 2, D], FP16, kind="ExternalInput").ap()
    bq = nc.dram_tensor("bq", [P, 2], F32, kind="ExternalInput").ap()
    bk = nc.dram_tensor("bk", [P, 2], F32, kind="ExternalInput").ap()
    out = nc.dram_tensor("out", [S, D], FP16, kind="ExternalOutput").ap()

    with tile.TileContext(nc) as tc:
        build_tile_kernel(nc, tc, xq, xk, xv, wq, wk, wv, wo, bq, bk, out)

    nc.compile()
    return nc


def build_tile_kernel(nc, tc, xq, xk, xv, wq, wk, wv, wo, bq, bk, out):
    from contextlib import ExitStack

    with ExitStack() as ctx:
        singles = ctx.enter_context(tc.tile_pool(name="singles", bufs=1))
        persist = ctx.enter_context(tc.tile_pool(name="persist", bufs=1))
        # PSUM: psA = scores slots (2 banks each), psB = chain slots (1 bank)
        psA = ctx.enter_context(tc.tile_pool(name="psA", bufs=3, space="PSUM"))
        psB = ctx.enter_context(tc.tile_pool(name="psB", bufs=2, space="PSUM"))
        xT_pool = ctx.enter_context(tc.tile_pool(name="xT", bufs=8))
        pT_pool = ctx.enter_context(tc.tile_pool(name="pT", bufs=6))
        norm_pool = ctx.enter_context(tc.tile_pool(name="norm", bufs=2))
        out_sb_pool = ctx.enter_context(tc.tile_pool(name="osb", bufs=2))

        # --- weight / bias tiles -------------------------------------------------
        w_q = singles.tile([P, 8, DC], FP16, tag="w_q")
        w_k = singles.tile([P, 8, DC], FP16, tag="w_k")
        w_v = singles.tile([P, 8, DC], FP16, tag="w_v")
        w_o = singles.tile([P, 2, D], FP16, tag="w_o")
        bq_t = singles.tile([P, 2], F32, tag="bq")
        bk_t = singles.tile([P, 2], F32, tag="bk")

        # --- persistent activations ---------------------------------------------
        qT = persist.tile([P, 2, S], FP16, tag="qT")    # [d'%128, pair, s]
        kT = persist.tile([P, 2, S], FP16, tag="kT")
        v_sb = persist.tile([P, 16, H * (DK + 1)], FP16, tag="v_sb")
        ctxn = persist.tile([P, 2, S], FP16, tag="ctxn")  # [c%128, pair, q]

        # --- DMA emission, ordered by first use ----------------------------------
        x_tiles = {}

        def dma_x(name, dram, sc, half):
            t = xT_pool.tile([P, 4, 512], FP16, tag="xc",
                             name=f"x{name}{sc}{half}")
            nc.sync.dma_start(
                out=t, in_=dram[:, 8 * sc + 4 * half:8 * sc + 4 * half + 4, :])
            x_tiles[(name, sc, half)] = t

        nc.sync.dma_start(out=bq_t, in_=bq)
        nc.sync.dma_start(out=bk_t, in_=bk)
        nc.sync.dma_start(out=w_k, in_=wk)
        dma_x("k", xk, 0, 0)
        dma_x("k", xk, 0, 1)
        nc.sync.dma_start(out=w_q, in_=wq)
        dma_x("q", xq, 0, 0)
        dma_x("q", xq, 0, 1)
        dma_x("k", xk, 1, 0)
        dma_x("k", xk, 1, 1)
        dma_x("k", xk, 2, 0)
        dma_x("k", xk, 2, 1)
        dma_x("k", xk, 3, 0)
        dma_x("k", xk, 3, 1)
        nc.sync.dma_start(out=w_v, in_=wv)
        dma_x("v", xv, 0, 0)
        dma_x("v", xv, 0, 1)
        dma_x("v", xv, 1, 0)
        dma_x("v", xv, 1, 1)
        dma_x("q", xq, 1, 0)
        dma_x("q", xq, 1, 1)
        dma_x("v", xv, 2, 0)
        dma_x("v", xv, 2, 1)
        dma_x("v", xv, 3, 0)
        dma_x("v", xv, 3, 1)
        nc.sync.dma_start(out=w_o, in_=wo)
        dma_x("q", xq, 2, 0)
        dma_x("q", xq, 2, 1)
        dma_x("q", xq, 3, 0)
        dma_x("q", xq, 3, 1)

        for h in range(H):  # ones column per head for rowsum-in-matmul
            nc.vector.memset(v_sb[:, :, h * 65 + 64:h * 65 + 65], 1.0)

        # PE pre-warm: the tensor engine runs at 0.65-1.2 GHz for the first
        # ~3us after idle. Dummy matmuls on scratch data ramp the clock while
        # the first input DMAs are in flight, and keep it up through the
        # DMA-bound window before the first Q projection.
        scr = singles.tile([P, 512], FP16, tag="scr")
        nc.vector.memset(scr[:, 0:512], 0.0)
        warm_n = [0]

        def warmup(n, nfree):
            warm_n[0] += 1
            warm = psB.tile([P, 512], F32, tag="ps1",
                            name=f"warm{warm_n[0]}")
            for _ in range(n):
                nc.tensor.matmul(warm[:, 0:nfree], lhsT=scr[:, 0:128],
                                 rhs=scr[:, 0:nfree])

        # --- unit emitters -------------------------------------------------------
        proj_accs = {}

        def qk_half(name, w_t, b_t, dest, sc, m, lo):
            # half of a Q/K projection m-chain: 4 matmuls N=512
            if lo == 0:
                pr = psB.tile([P, 512], F32, tag="ps1",
                              name=f"pr_{name}_{sc}_{m}")
                proj_accs[(name, sc, m)] = pr
            else:
                pr = proj_accs.pop((name, sc, m))
            for dt in range(4 * lo, 4 * lo + 4):
                xc = x_tiles[(name, sc, dt // 4)]
                nc.tensor.matmul(
                    pr,
                    lhsT=w_t[:, dt, 128 * m:128 * (m + 1)],
                    rhs=xc[:, dt % 4, :],
                    start=(dt == 0), stop=(dt == 7))
            if lo:
                nc.vector.tensor_scalar_add(
                    dest[:, m, 512 * sc:512 * (sc + 1)], pr, b_t[:, m:m + 1])

        def qk_proj(name, w_t, b_t, dest, sc, m):
            qk_half(name, w_t, b_t, dest, sc, m, 0)
            qk_half(name, w_t, b_t, dest, sc, m, 1)

        def v_proj(sc, st):
            # one st-chain of the V projection: 8 matmuls N=256
            pv = psB.tile([P, DC], F32, tag="ps1", name=f"pv_{sc}_{st}")
            for dt in range(8):
                xc = x_tiles[("v", sc, dt // 4)]
                nc.tensor.matmul(
                    pv,
                    lhsT=xc[:, dt % 4, 128 * st:128 * (st + 1)],
                    rhs=w_v[:, dt, :],
                    start=(dt == 0), stop=(dt == 7))
            kt = 4 * sc + st
            nc.vector.tensor_copy(
                v_sb[:, kt, :].rearrange("p (h c) -> p h c", h=H)[:, :, 0:DK],
                pv.rearrange("p (h c) -> p h c", c=DK))

        pT_tiles = {}

        def s_unit(qc, pr_i, half, kg):
            # scores for one head over a 256-wide k slab + exp: 2 mm + 1 ACT
            h = 2 * pr_i + half
            if (qc, h) not in pT_tiles:
                pT_tiles[(qc, h)] = pT_pool.tile(
                    [P, 16, 512], FP16, tag="pT", name=f"pT_{qc}_{h}")
            pT_h = pT_tiles[(qc, h)]
            qsl = slice(512 * qc, 512 * (qc + 1))
            rows = slice(64 * half, 64 * half + 64)
            sc_ = psA.tile([P, 2, 512], F32, tag="sc",
                           name=f"sc_{qc}_{pr_i}_{half}_{kg}")
            for khi in range(2):
                kt = 2 * kg + khi
                nc.tensor.matmul(sc_[:, khi, :],
                                 lhsT=kT[rows, pr_i, 128 * kt:128 * (kt + 1)],
                                 rhs=qT[rows, pr_i, qsl])
            nc.scalar.activation(
                pT_h[:, 2 * kg:2 * kg + 2, :].rearrange("p a b -> p (a b)"),
                sc_.rearrange("p a b -> p (a b)"),
                mybir.ActivationFunctionType.Exp)

        ctx_accs = {}

        def ctx_half(qc, h, lo):
            # half of a ctx chain: 8 matmuls; second half finishes + normalizes
            qsl = slice(512 * qc, 512 * (qc + 1))
            pr_i, hp = divmod(h, 2)
            if lo == 0:
                acc = psB.tile([P, 512], F32, tag="ps1", name=f"cp_{qc}_{h}")
                ctx_accs[(qc, h)] = acc
            else:
                acc = ctx_accs.pop((qc, h))
            pT_h = pT_tiles[(qc, h)]
            for kt in range(8 * lo, 8 * lo + 8):
                nc.tensor.matmul(
                    acc[0:65, :],
                    lhsT=v_sb[:, kt, 65 * h:65 * h + 65],
                    rhs=pT_h[:, kt, :],
                    start=(kt == 0), stop=(kt == 15))
            if lo == 0:
                return
            del pT_tiles[(qc, h)]
            # normalize: ctx_n = ctx * bcast(1/rowsum)
            rs = norm_pool.tile([1, 512], F32, tag="rs", name=f"rs_{qc}_{h}")
            nc.vector.tensor_copy(rs, acc[64:65, :])
            rc = norm_pool.tile([1, 512], F32, tag="rc", name=f"rc_{qc}_{h}")
            nc.vector.reciprocal_approx_fast(rc, rs)
            bc = norm_pool.tile([64, 512], F32, tag="bc", name=f"bc_{qc}_{h}")
            nc.gpsimd.partition_broadcast(bc, rc[0:1, :], channels=64)
            nc.vector.tensor_mul(
                ctxn[64 * hp:64 * hp + 64, pr_i, qsl], acc[0:64, :], bc)

        def outproj(qc, stl):
            st = 4 * qc + stl
            ob = out_sb_pool.tile([P, D], FP16, tag="ob", name=f"ob_{st}")
            for jc in range(2):
                op = psB.tile([P, 512], F32, tag="ps1", name=f"op_{st}_{jc}")
                for ct in range(2):
                    nc.tensor.matmul(
                        op,
                        lhsT=ctxn[:, ct, 128 * st:128 * (st + 1)],
                        rhs=w_o[:, ct, 512 * jc:512 * (jc + 1)],
                        start=(ct == 0), stop=(ct == 1))
                nc.vector.tensor_copy(ob[:, 512 * jc:512 * (jc + 1)], op)
            nc.sync.dma_start(out=out[128 * st:128 * (st + 1), :], in_=ob)

        # --- schedule ------------------------------------------------------------
        # prologue: K chunk 0 + Q chunk 0 pair 0 (minimum for first S unit)
        warmup(14, 256)          # ramp while w_k + xk0 stream in
        qk_proj("k", w_k, bk_t, kT, 0, 0)
        qk_proj("k", w_k, bk_t, kT, 0, 1)
        warmup(20, 128)          # hold clock while w_q + xq0 stream in
        qk_proj("q", w_q, bq_t, qT, 0, 0)

        # filler inventory per 16-tick block, (cycles, emit) pairs.
        # CT halves must stay adjacent (open PSUM chain; only S units between).
        def F(cyc, fn, *a):
            return (cyc, lambda: fn(*a))

        def KPu(sc, m):
            return [F(2048, qk_half, "k", w_k, bk_t, kT, sc, m, 0),
                    F(2048, qk_half, "k", w_k, bk_t, kT, sc, m, 1)]

        def QPu(sc, m):
            return [F(2048, qk_half, "q", w_q, bq_t, qT, sc, m, 0),
                    F(2048, qk_half, "q", w_q, bq_t, qT, sc, m, 1)]

        def VPu(sc, st):
            return F(2048, v_proj, sc, st)

        def CTu(qc, h):
            return [F(4096, ctx_half, qc, h, 0), F(4096, ctx_half, qc, h, 1)]

        def OPu(qc, stl):
            return F(2048, outproj, qc, stl)

        blocks = [
            # b0 (qc0,pr0)
            KPu(1, 0) + KPu(1, 1) + KPu(2, 0) + KPu(2, 1) + KPu(3, 0)
            + KPu(3, 1) + QPu(0, 1),
            # b1 (qc0,pr1)
            [VPu(0, st) for st in range(4)] + QPu(1, 0) + QPu(1, 1)
            + [VPu(1, st) for st in range(4)],
            # b2 (qc1,pr0)
            [VPu(2, st) for st in range(4)] + [VPu(3, st) for st in range(4)]
            + CTu(0, 0),
            # b3 (qc1,pr1)
            CTu(0, 1) + CTu(0, 2) + CTu(0, 3) + QPu(2, 0) + QPu(2, 1),
            # b4 (qc2,pr0)
            [OPu(0, stl) for stl in range(4)] + CTu(1, 0) + CTu(1, 1)
            + CTu(1, 2),
            # b5 (qc2,pr1)
            CTu(1, 3) + CTu(2, 0) + [OPu(1, stl) for stl in range(4)]
            + QPu(3, 0),
            # b6 (qc3,pr0)
            QPu(3, 1) + CTu(2, 1) + CTu(2, 2) + CTu(2, 3),
            # b7 (qc3,pr1): S units run all a-halves then all b-halves
            [OPu(2, stl) for stl in range(4)] + CTu(3, 0) + CTu(3, 1)
            + CTu(3, 2),
        ]

        # global pacer: one continuous S stream; leftover fillers flow across
        # block boundaries so no burst ever starves the scalar engine of exp.
        queue = []
        done = 0
        cum = 0
        for bi in range(8):
            qc, pr_i = divmod(bi, 2)
            queue.extend(blocks[bi])
            btotal = sum(c for c, _ in blocks[bi])
            for lt in range(16):
                if bi == 7:
                    half, kg = divmod(lt, 8)
                else:
                    kg, half = divmod(lt, 2)
                s_unit(qc, pr_i, half, kg)
                target = cum + btotal * min(lt + 2, 16) // 16
                while queue and done < target:
                    cyc, fn = queue.pop(0)
                    fn()
                    done += cyc
            cum += btotal
        for cyc, fn in queue:
            fn()

        # epilogue: last head, ctx chain + norm + outproj pipelined by q-halves
        # (separate PSUM accs per half so the halves have no false deps)
        pT_h = pT_tiles.pop((3, 3))
        qb = 512 * 3
        haccs = {}

        def last_chain(qh):
            hacc = psB.tile([P, 256], F32, tag="ps1", name=f"cpz{qh}")
            haccs[qh] = hacc
            for kt in range(16):
                nc.tensor.matmul(
                    hacc[0:65, :],
                    lhsT=v_sb[:, kt, 65 * 3:65 * 3 + 65],
                    rhs=pT_h[:, kt, 256 * qh:256 * qh + 256],
                    start=(kt == 0), stop=(kt == 15))

        def last_norm(qh):
            hacc = haccs[qh]
            rs = norm_pool.tile([1, 256], F32, tag="rs", name=f"rsz{qh}")
            nc.vector.tensor_copy(rs, hacc[64:65, :])
            rc = norm_pool.tile([1, 256], F32, tag="rc", name=f"rcz{qh}")
            nc.vector.reciprocal_approx_fast(rc, rs)
            bc = norm_pool.tile([64, 256], F32, tag="bc", name=f"bcz{qh}")
            nc.gpsimd.partition_broadcast(bc, rc[0:1, :], channels=64)
            nc.vector.tensor_mul(
                ctxn[64:128, 1, qb + 256 * qh:qb + 256 * qh + 256],
                hacc[0:64, :], bc)

        last_chain(0)
        last_norm(0)    # DVE/gpsimd chain overlaps half-1 ctx matmuls
        last_chain(1)
        outproj(3, 0)
        outproj(3, 1)
        last_norm(1)
        outproj(3, 2)
        outproj(3, 3)


def make_in_maps(Q_input, K_input, V_input, Wq, bq, Wk, bk, Wv, Wo):
    scale = 0.125  # 1/sqrt(64), exact power of two

    def prep_x(x):
        # [S, D] -> xp[p, 8*sc + t, s] = x.T[128*t + p, 512*sc + s]
        xt = x.T.astype(np.float16)                    # [1024, 2048]
        a = xt.reshape(8, P, 4, 512)                   # [t, p, sc, s]
        return np.ascontiguousarray(a.transpose(1, 2, 0, 3)).reshape(P, 32, 512)

    def prep_w(wt):
        # [1024, DC] -> [p, t, c]
        return np.ascontiguousarray(
            wt.reshape(8, P, -1).transpose(1, 0, 2)).astype(np.float16)

    xp = {}
    for b in range(2):
        xp[("q", b)] = prep_x(Q_input[b])
        xp[("k", b)] = prep_x(K_input[b])
        xp[("v", b)] = prep_x(V_input[b])
    in_maps = []
    for c in range(NCORES):
        b, g = divmod(c, 4)
        sl = slice(DC * g, DC * (g + 1))
        in_maps.append({
            "xq": xp[("q", b)],
            "xk": xp[("k", b)],
            "xv": xp[("v", b)],
            "wq": prep_w(np.ascontiguousarray(Wq[sl, :].T) * scale),
            "wk": prep_w(np.ascontiguousarray(Wk[sl, :].T)),
            "wv": prep_w(np.ascontiguousarray(Wv[sl, :].T)),
            "wo": np.ascontiguousarray(
                Wo[:, sl].T.reshape(2, P, D).transpose(1, 0, 2)
            ).astype(np.float16),
            "bq": (bq[sl] * scale).reshape(2, P).T.astype(np.float32).copy(),
            "bk": bk[sl].reshape(2, P).T.astype(np.float32).copy(),
        })
    return in_maps


def kernel(Q_input, K_input, V_input, Wq, bq, Wk, bk, Wv, bv, Wo, bo):
    if "nc" not in _cached:
        _cached["nc"] = build_program()
    nc = _cached["nc"]

    in_maps = make_in_maps(Q_input, K_input, V_input, Wq, bq, Wk, bk, Wv, Wo)
    res = run_bass_kernel_spmd(nc, in_maps, list(range(NCORES))).results
    outs = [res[c]["out"] for c in range(NCORES)]

    const = (bv.astype(np.float32) @ Wo.T.astype(np.float32)) + bo
    full = np.empty((2, S, D), np.float32)
    for b in range(2):
        acc = outs[4 * b].astype(np.float32)
        for g in range(1, 4):
            acc += outs[4 * b + g].astype(np.float32)
        full[b] = acc + const
    return full


# revision 30
# speedup vs baseline: 1.0146x; 1.0081x over previous
"""MultiHeadAttention Trainium2 kernel (8 NeuronCores).

Sharding: data-parallel over batch (2) x tensor-parallel over heads (16/4=4
head groups). Core c handles batch b = c//4 and heads 4g..4g+4 (g = c%4),
i.e. a 256-wide column slice of Wq/Wk/Wv and the matching row slice of Wo.
Each core computes a full [2048, 1024] partial output (its heads' ctx @ Wo
row-slice); the host sums the 4 partials per batch and adds the bias terms.

v3: fine-grained software pipeline. The kernel interleaves scores+exp units
(2 matmuls + 1 ACT) with projection / ctx / out-proj chain fillers at ~1 us
granularity so the tensor engine never idles (PE p-state ramp makes every
idle gap cost double) and the scalar engine (exp, ~143 us total) always has
backlog. Host pre-arranges x and weights into the exact SBUF tile layouts so
every DMA is large contiguous descriptors, ordered by first use. Output is
written fp16 (host accumulates partials in f32). 1/sqrt(dk) folded into
Wq/bq.

Per-core dataflow (all matmul operands fp16, fp32 PSUM):
  Q.T, K.T = W.T @ xT + b (per-partition bias)     [d'=256, s] pair-packed
  V        = xT.T @ WvT (no bias; folded on host)  [s, c] + ones col/head
  scores.T = K.T_h.T @ Q.T_h (per 256-wide k slab) [k, q] in PSUM
  P.T      = exp(scores.T) on ACT, fp16            [k, q] SBUF
  ctx.T|r  = [V_h | 1].T @ P.T (M=65, fused rowsum), 16-kt chain in PSUM
  ctx_n    = ctx.T * broadcast(1/r)                [c, q] pair-packed fp16
  out_u    = ctx_n.T @ WoT                         [s, 1024] -> DRAM fp16
"""

import numpy as np

import concourse.bass as bass
import concourse.mybir as mybir
import concourse.tile as tile
from concourse import bacc
from concourse.bass_utils import run_bass_kernel_spmd

S = 2048          # sequence length
D = 1024          # model dim
DC = 256          # d' columns per core (4 heads x 64)
H = 4             # heads per core
DK = 64           # head dim
P = 128
F32 = mybir.dt.float32
FP16 = mybir.dt.float16
NCORES = 8

_cached = {}


def build_program():
    nc = bacc.Bacc("TRN2", target_bir_lowering=False, debug=False,
                   num_devices=NCORES)

    # x pre-arranged on host: xp[p, 8*sc + t, s] = x.T[128*t + p, 512*sc + s]
    xq = nc.dram_tensor("xq", [P, 32, 512], FP16, kind="ExternalInput").ap()
    xk = nc.dram_tensor("xk", [P, 32, 512], FP16, kind="ExternalInput").ap()
    xv = nc.dram_tensor("xv", [P, 32, 512], FP16, kind="ExternalInput").ap()
    # weights pre-arranged: w[p, t, c] = W.T[128*t + p, c]
    wq = nc.dram_tensor("wq", [P, 8, DC], FP16, kind="ExternalInput").ap()
    wk = nc.dram_tensor("wk", [P, 8, DC], FP16, kind="ExternalInput").ap()
    wv = nc.dram_tensor("wv", [P, 8, DC], FP16, kind="ExternalInput").ap()
    wo = nc.dram_tensor("wo", [P, 2, D], FP16, kind="ExternalInput").ap()
    bq = nc.dram_tensor("bq", [P, 2], F32, kind="ExternalInput").ap()
    bk = nc.dram_tensor("bk", [P, 2], F32, kind="ExternalInput").ap()
    out = nc.dram_tensor("out", [S, D], FP16, kind="ExternalOutput").ap()

    with tile.TileContext(nc) as tc:
        build_tile_kernel(nc, tc, xq, xk, xv, wq, wk, wv, wo, bq, bk, out)

    nc.compile()
    return nc


def build_tile_kernel(nc, tc, xq, xk, xv, wq, wk, wv, wo, bq, bk, out):
    from contextlib import ExitStack

    with ExitStack() as ctx:
        singles = ctx.enter_context(tc.tile_pool(name="singles", bufs=1))
        persist = ctx.enter_context(tc.tile_pool(name="persist", bufs=1))
        # PSUM: psA = scores slots (2 banks each), psB = chain slots (1 bank)
        psA = ctx.enter_context(tc.tile_pool(name="psA", bufs=3, space="PSUM"))
        psB = ctx.enter_context(tc.tile_pool(name="psB", bufs=2, space="PSUM"))
        xT_pool = ctx.enter_context(tc.tile_pool(name="xT", bufs=8))
        pT_pool = ctx.enter_context(tc.tile_pool(name="pT", bufs=6))
        norm_pool = ctx.enter_context(tc.tile_pool(name="norm", bufs=2))
        out_sb_pool = ctx.enter_context(tc.tile_pool(name="osb", bufs=2))

        # --- weight / bias tiles -------------------------------------------------
        w_q = singles.tile([P, 8, DC], FP16, tag="w_q")
        w_k = singles.tile([P, 8, DC], FP16, tag="w_k")
        w_v = singles.tile([P, 8, DC], FP16, tag="w_v")
        w_o = singles.tile([P, 2, D], FP16, tag="w_o")
        bq_t = singles.tile([P, 2], F32, tag="bq")
        bk_t = singles.tile([P, 2], F32, tag="bk")

        # --- persistent activations ---------------------------------------------
        qT = persist.tile([P, 2, S], FP16, tag="qT")    # [d'%128, pair, s]
        kT = persist.tile([P, 2, S], FP16, tag="kT")
        v_sb = persist.tile([P, 16, H * (DK + 1)], FP16, tag="v_sb")
        ctxn = persist.tile([P, 2, S], FP16, tag="ctxn")  # [c%128, pair, q]

        # --- DMA emission, ordered by first use ----------------------------------
        x_tiles = {}

        def dma_x(name, dram, sc, half):
            t = xT_pool.tile([P, 4, 512], FP16, tag="xc",
                             name=f"x{name}{sc}{half}")
            nc.sync.dma_start(
                out=t, in_=dram[:, 8 * sc + 4 * half:8 * sc + 4 * half + 4, :])
            x_tiles[(name, sc, half)] = t

        nc.sync.dma_start(out=bq_t, in_=bq)
        nc.sync.dma_start(out=bk_t, in_=bk)
        nc.sync.dma_start(out=w_k, in_=wk)
        dma_x("k", xk, 0, 0)
        dma_x("k", xk, 0, 1)
        nc.sync.dma_start(out=w_q, in_=wq)
        dma_x("q", xq, 0, 0)
        dma_x("q", xq, 0, 1)
        dma_x("k", xk, 1, 0)
        dma_x("k", xk, 1, 1)
        dma_x("k", xk, 2, 0)
        dma_x("k", xk, 2, 1)
        dma_x("k", xk, 3, 0)
        dma_x("k", xk, 3, 1)
        nc.sync.dma_start(out=w_v, in_=wv)
        dma_x("v", xv, 0, 0)
        dma_x("v", xv, 0, 1)
        dma_x("v", xv, 1, 0)
        dma_x("v", xv, 1, 1)
        dma_x("q", xq, 1, 0)
        dma_x("q", xq, 1, 1)
        dma_x("v", xv, 2, 0)
        dma_x("v", xv, 2, 1)
        dma_x("v", xv, 3, 0)
        dma_x("v", xv, 3, 1)
        nc.sync.dma_start(out=w_o, in_=wo)
        dma_x("q", xq, 2, 0)
        dma_x("q", xq, 2, 1)
        dma_x("q", xq, 3, 0)
        dma_x("q", xq, 3, 1)

        for h in range(H):  # ones column per head for rowsum-in-matmul
            nc.vector.memset(v_sb[:, :, h * 65 + 64:h * 65 + 65], 1.0)

        # PE pre-warm: the tensor engine runs at 0.65-1.2 GHz for the first
        # ~3us after idle. Dummy matmuls on scratch data ramp the clock while
        # the first input DMAs are in flight, and keep it up through the
        # DMA-bound window before the first Q projection.
        scr = singles.tile([P, 512], FP16, tag="scr")
        nc.vector.memset(scr[:, 0:512], 0.0)
        warm_n = [0]

        def warmup(n, nfree):
            warm_n[0] += 1
            warm = psB.tile([P, 512], F32, tag="ps1",
                            name=f"warm{warm_n[0]}")
            for _ in range(n):
                nc.tensor.matmul(warm[:, 0:nfree], lhsT=scr[:, 0:128],
                                 rhs=scr[:, 0:nfree])

        # --- unit emitters -------------------------------------------------------
        proj_accs = {}

        def qk_half(name, w_t, b_t, dest, sc, m, lo):
            # half of a Q/K projection m-chain: 4 matmuls N=512
            if lo == 0:
                pr = psB.tile([P, 512], F32, tag="ps1",
                              name=f"pr_{name}_{sc}_{m}")
                proj_accs[(name, sc, m)] = pr
            else:
                pr = proj_accs.pop((name, sc, m))
            for dt in range(4 * lo, 4 * lo + 4):
                xc = x_tiles[(name, sc, dt // 4)]
                nc.tensor.matmul(
                    pr,
                    lhsT=w_t[:, dt, 128 * m:128 * (m + 1)],
                    rhs=xc[:, dt % 4, :],
                    start=(dt == 0), stop=(dt == 7))
            if lo:
                nc.vector.tensor_scalar_add(
                    dest[:, m, 512 * sc:512 * (sc + 1)], pr, b_t[:, m:m + 1])

        def qk_proj(name, w_t, b_t, dest, sc, m):
            qk_half(name, w_t, b_t, dest, sc, m, 0)
            qk_half(name, w_t, b_t, dest, sc, m, 1)

        def v_proj(sc, st):
            # one st-chain of the V projection: 8 matmuls N=256
            pv = psB.tile([P, DC], F32, tag="ps1", name=f"pv_{sc}_{st}")
            for dt in range(8):
                xc = x_tiles[("v", sc, dt // 4)]
                nc.tensor.matmul(
                    pv,
                    lhsT=xc[:, dt % 4, 128 * st:128 * (st + 1)],
                    rhs=w_v[:, dt, :],
                    start=(dt == 0), stop=(dt == 7))
            kt = 4 * sc + st
            nc.vector.tensor_copy(
                v_sb[:, kt, :].rearrange("p (h c) -> p h c", h=H)[:, :, 0:DK],
                pv.rearrange("p (h c) -> p h c", c=DK))

        pT_tiles = {}

        def s_unit(qc, pr_i, half, kg):
            # scores for one head over a 256-wide k slab + exp: 2 mm + 1 ACT
            h = 2 * pr_i + half
            if (qc, h) not in pT_tiles:
                pT_tiles[(qc, h)] = pT_pool.tile(
                    [P, 16, 512], FP16, tag="pT", name=f"pT_{qc}_{h}")
            pT_h = pT_tiles[(qc, h)]
            qsl = slice(512 * qc, 512 * (qc + 1))
            rows = slice(64 * half, 64 * half + 64)
            sc_ = psA.tile([P, 2, 512], F32, tag="sc",
                           name=f"sc_{qc}_{pr_i}_{half}_{kg}")
            for khi in range(2):
                kt = 2 * kg + khi
                nc.tensor.matmul(sc_[:, khi, :],
                                 lhsT=kT[rows, pr_i, 128 * kt:128 * (kt + 1)],
                                 rhs=qT[rows, pr_i, qsl])
            nc.scalar.activation(
                pT_h[:, 2 * kg:2 * kg + 2, :].rearrange("p a b -> p (a b)"),
                sc_.rearrange("p a b -> p (a b)"),
                mybir.ActivationFunctionType.Exp)

        ctx_accs = {}

        def ctx_half(qc, h, lo):
            # half of a ctx chain: 8 matmuls; second half finishes + normalizes
            qsl = slice(512 * qc, 512 * (qc + 1))
            pr_i, hp = divmod(h, 2)
            if lo == 0:
                acc = psB.tile([P, 512], F32, tag="ps1", name=f"cp_{qc}_{h}")
                ctx_accs[(qc, h)] = acc
            else:
                acc = ctx_accs.pop((qc, h))
            pT_h = pT_tiles[(qc, h)]
            for kt in range(8 * lo, 8 * lo + 8):
                nc.tensor.matmul(
                    acc[0:65, :],
                    lhsT=v_sb[:, kt, 65 * h:65 * h + 65],
                    rhs=pT_h[:, kt, :],
                    start=(kt == 0), stop=(kt == 15))
            if lo == 0:
                return
            del pT_tiles[(qc, h)]
            # normalize: ctx_n = ctx * bcast(1/rowsum)
            rs = norm_pool.tile([1, 512], F32, tag="rs", name=f"rs_{qc}_{h}")
            nc.vector.tensor_copy(rs, acc[64:65, :])
            rc = norm_pool.tile([1, 512], F32, tag="rc", name=f"rc_{qc}_{h}")
            nc.vector.reciprocal_approx_fast(rc, rs)
            bc = norm_pool.tile([64, 512], F32, tag="bc", name=f"bc_{qc}_{h}")
            nc.gpsimd.partition_broadcast(bc, rc[0:1, :], channels=64)
            nc.vector.tensor_mul(
                ctxn[64 * hp:64 * hp + 64, pr_i, qsl], acc[0:64, :], bc)

        def outproj(qc, stl):
            st = 4 * qc + stl
            ob = out_sb_pool.tile([P, D], FP16, tag="ob", name=f"ob_{st}")
            for jc in range(2):
                op = psB.tile([P, 512], F32, tag="ps1", name=f"op_{st}_{jc}")
                for ct in range(2):
                    nc.tensor.matmul(
                        op,
                        lhsT=ctxn[:, ct, 128 * st:128 * (st + 1)],
                        rhs=w_o[:, ct, 512 * jc:512 * (jc + 1)],
                        start=(ct == 0), stop=(ct == 1))
                if qc == 3 and jc == 1:
                    # scalar engine is idle after the last exp; offload CASTs
                    nc.scalar.activation(
                        ob[:, 512 * jc:512 * (jc + 1)], op,
                        mybir.ActivationFunctionType.Copy)
                else:
                    nc.vector.tensor_copy(ob[:, 512 * jc:512 * (jc + 1)], op)
            nc.sync.dma_start(out=out[128 * st:128 * (st + 1), :], in_=ob)

        # --- schedule ------------------------------------------------------------
        # prologue: K chunk 0 + Q chunk 0 pair 0 (minimum for first S unit)
        warmup(14, 256)          # ramp while w_k + xk0 stream in
        qk_proj("k", w_k, bk_t, kT, 0, 0)
        qk_proj("k", w_k, bk_t, kT, 0, 1)
        warmup(20, 128)          # hold clock while w_q + xq0 stream in
        qk_proj("q", w_q, bq_t, qT, 0, 0)

        # filler inventory per 16-tick block, (cycles, emit) pairs.
        # CT halves must stay adjacent (open PSUM chain; only S units between).
        def F(cyc, fn, *a):
            return (cyc, lambda: fn(*a))

        def KPu(sc, m):
            return [F(2048, qk_half, "k", w_k, bk_t, kT, sc, m, 0),
                    F(2048, qk_half, "k", w_k, bk_t, kT, sc, m, 1)]

        def QPu(sc, m):
            return [F(2048, qk_half, "q", w_q, bq_t, qT, sc, m, 0),
                    F(2048, qk_half, "q", w_q, bq_t, qT, sc, m, 1)]

        def VPu(sc, st):
            return F(2048, v_proj, sc, st)

        def CTu(qc, h):
            return [F(4096, ctx_half, qc, h, 0), F(4096, ctx_half, qc, h, 1)]

        def OPu(qc, stl):
            return F(2048, outproj, qc, stl)

        blocks = [
            # b0 (qc0,pr0)
            KPu(1, 0) + KPu(1, 1) + KPu(2, 0) + KPu(2, 1) + KPu(3, 0)
            + KPu(3, 1) + QPu(0, 1),
            # b1 (qc0,pr1)
            [VPu(0, st) for st in range(4)] + QPu(1, 0) + QPu(1, 1)
            + [VPu(1, st) for st in range(4)],
            # b2 (qc1,pr0)
            [VPu(2, st) for st in range(4)] + [VPu(3, st) for st in range(4)]
            + CTu(0, 0),
            # b3 (qc1,pr1)
            CTu(0, 1) + CTu(0, 2) + CTu(0, 3) + QPu(2, 0) + QPu(2, 1),
            # b4 (qc2,pr0)
            [OPu(0, stl) for stl in range(4)] + CTu(1, 0) + CTu(1, 1)
            + CTu(1, 2),
            # b5 (qc2,pr1)
            CTu(1, 3) + CTu(2, 0) + [OPu(1, stl) for stl in range(4)]
            + QPu(3, 0),
            # b6 (qc3,pr0)
            QPu(3, 1) + CTu(2, 1) + CTu(2, 2) + CTu(2, 3),
            # b7 (qc3,pr1): S units run all a-halves then all b-halves
            [OPu(2, stl) for stl in range(4)] + CTu(3, 0) + CTu(3, 1)
            + CTu(3, 2),
        ]

        # global pacer: one continuous S stream; leftover fillers flow across
        # block boundaries so no burst ever starves the scalar engine of exp.
        queue = []
        done = 0
        cum = 0
        for bi in range(8):
            qc, pr_i = divmod(bi, 2)
            queue.extend(blocks[bi])
            btotal = sum(c for c, _ in blocks[bi])
            for lt in range(16):
                if bi == 7:
                    half, kg = divmod(lt, 8)
                else:
                    kg, half = divmod(lt, 2)
                s_unit(qc, pr_i, half, kg)
                target = cum + btotal * min(lt + 2, 16) // 16
                while queue and done < target:
                    cyc, fn = queue.pop(0)
                    fn()
                    done += cyc
            cum += btotal
        for cyc, fn in queue:
            fn()

        # epilogue: last head, ctx chain + norm + outproj pipelined by q-halves
        # (separate PSUM accs per half so the halves have no false deps)
        pT_h = pT_tiles.pop((3, 3))
        qb = 512 * 3
        haccs = {}

        def last_chain(qh):
            hacc = psB.tile([P, 256], F32, tag="ps1", name=f"cpz{qh}")
            haccs[qh] = hacc
            for kt in range(16):
                nc.tensor.matmul(
                    hacc[0:65, :],
                    lhsT=v_sb[:, kt, 65 * 3:65 * 3 + 65],
                    rhs=pT_h[:, kt, 256 * qh:256 * qh + 256],
                    start=(kt == 0), stop=(kt == 15))

        def last_norm(qh):
            hacc = haccs[qh]
            rs = norm_pool.tile([1, 256], F32, tag="rs", name=f"rsz{qh}")
            nc.vector.tensor_copy(rs, hacc[64:65, :])
            rc = norm_pool.tile([1, 256], F32, tag="rc", name=f"rcz{qh}")
            nc.vector.reciprocal_approx_fast(rc, rs)
            bc = norm_pool.tile([64, 256], F32, tag="bc", name=f"bcz{qh}")
            nc.gpsimd.partition_broadcast(bc, rc[0:1, :], channels=64)
            nc.vector.tensor_mul(
                ctxn[64:128, 1, qb + 256 * qh:qb + 256 * qh + 256],
                hacc[0:64, :], bc)

        last_chain(0)
        last_norm(0)    # DVE/gpsimd chain overlaps half-1 ctx matmuls
        last_chain(1)
        last_norm(1)
        outproj(3, 0)
        outproj(3, 1)
        outproj(3, 2)
        outproj(3, 3)


def make_in_maps(Q_input, K_input, V_input, Wq, bq, Wk, bk, Wv, Wo):
    scale = 0.125  # 1/sqrt(64), exact power of two

    def prep_x(x):
        # [S, D] -> xp[p, 8*sc + t, s] = x.T[128*t + p, 512*sc + s]
        xt = x.T.astype(np.float16)                    # [1024, 2048]
        a = xt.reshape(8, P, 4, 512)                   # [t, p, sc, s]
        return np.ascontiguousarray(a.transpose(1, 2, 0, 3)).reshape(P, 32, 512)

    def prep_w(wt):
        # [1024, DC] -> [p, t, c]
        return np.ascontiguousarray(
            wt.reshape(8, P, -1).transpose(1, 0, 2)).astype(np.float16)

    xp = {}
    for b in range(2):
        xp[("q", b)] = prep_x(Q_input[b])
        xp[("k", b)] = prep_x(K_input[b])
        xp[("v", b)] = prep_x(V_input[b])
    in_maps = []
    for c in range(NCORES):
        b, g = divmod(c, 4)
        sl = slice(DC * g, DC * (g + 1))
        in_maps.append({
            "xq": xp[("q", b)],
            "xk": xp[("k", b)],
            "xv": xp[("v", b)],
            "wq": prep_w(np.ascontiguousarray(Wq[sl, :].T) * scale),
            "wk": prep_w(np.ascontiguousarray(Wk[sl, :].T)),
            "wv": prep_w(np.ascontiguousarray(Wv[sl, :].T)),
            "wo": np.ascontiguousarray(
                Wo[:, sl].T.reshape(2, P, D).transpose(1, 0, 2)
            ).astype(np.float16),
            "bq": (bq[sl] * scale).reshape(2, P).T.astype(np.float32).copy(),
            "bk": bk[sl].reshape(2, P).T.astype(np.float32).copy(),
        })
    return in_maps


def kernel(Q_input, K_input, V_input, Wq, bq, Wk, bk, Wv, bv, Wo, bo):
    if "nc" not in _cached:
        _cached["nc"] = build_program()
    nc = _cached["nc"]

    in_maps = make_in_maps(Q_input, K_input, V_input, Wq, bq, Wk, bk, Wv, Wo)
    res = run_bass_kernel_spmd(nc, in_maps, list(range(NCORES))).results
    outs = [res[c]["out"] for c in range(NCORES)]

    const = (bv.astype(np.float32) @ Wo.T.astype(np.float32)) + bo
    full = np.empty((2, S, D), np.float32)
    for b in range(2):
        acc = outs[4 * b].astype(np.float32)
        for g in range(1, 4):
            acc += outs[4 * b + g].astype(np.float32)
        full[b] = acc + const
    return full
